# revision 2
# baseline (speedup 1.0000x reference)
"""Trainium2 Bass kernel for nn_CrossAttention2 (8 cores, data-parallel over batch).

V3: fp16 IO/compute, weights DMA'd once, software-pipelined bodies.

Reference computation (per batch element b, one NeuronCore each):
    q = Wq @ x_b + bq          # [512, 1024]   x_b = x[b].reshape(512, 32*32)
    k = Wk @ c_b + bk
    v = Wv @ c_b + bv
    per head h (8 heads x 64 dim):
        S_h = (Q_h^T @ K_h) / 8
        E_h = exp(S_h); r_h[q] = 1/sum_k E_h[q,k]
        out_h = (V_h*r_h) @ E_h        # contraction over the QUERY axis
                                       # (faithful to the module's quirk)
    y_b = Wo @ concat(out_h) + bo

Pipeline (ScalarE's 64-exp stream is the per-body floor at ~79us; everything
else hides under it):
  - per body: ONE 2MB input DMA (ctx|x, fp16) on the SP HWDGE queue, issued a
    full body ahead; ONE 1MB output DMA on the Activation HWDGE queue.
  - weights/biases: single fp16 DMA at NEFF start, reused by every body.
  - emission order feeds ScalarE back-to-back: kq0/s0 of body n+1 are emitted
    before body n's output projection so the exp stream never drains at the
    body boundary; vproj hides under s1's exps; AV(h) lags one head.
  - PSUM: scores pool (2x2 banks), proj pool (1x2), AV accumulator (1x2).
  - Q/K/Y biases: DVE tensor_scalar_add during PSUM->SBUF eviction (f32
    column scalars, converted once on-chip from the fp16 blob); V's bias
    (broadcast along partitions) stays a rank-1 PE update.
  - softmax skips max-subtraction (scores are O(+-8) after the /8 scale is
    folded into Wq; exp stays in f32 PSUM range), normalization is folded
    into V^T rows (16x fewer elements than scaling probabilities).
"""

import os
import numpy as np
from contextlib import ExitStack

import concourse.bass as bass
from concourse import bacc
import concourse.tile as tile
from concourse import mybir
from concourse.bass_utils import run_bass_kernel_spmd

F32 = mybir.dt.float32
F16 = mybir.dt.float16
F32R = mybir.dt.float32r
ATT_DT = {"f16": F16, "f32r": F32R}[os.environ.get("ATT_DT", "f16")]

BS, C, H, W = 8, 512, 32, 32
HW = H * W
N_HEADS, DIM_HEAD = 8, 64
INNER = N_HEADS * DIM_HEAD
N_CORES = 8

OFF_WQ = 0
OFF_WK = 2048
OFF_WV = 4096
OFF_WO = 6144
OFF_BVR = 8192
OFF_ONES = 8704
OFF_BIAS = 9216     # fp16 [128,12] bq/8|bk|bo column-form biases
WBLOB_F = 9228
OFF_CTX = 0
OFF_X = 4096
XBLOB_F = 8192


def make_pools(ctx: ExitStack, tc: tile.TileContext):
    p = {}
    p["w"] = ctx.enter_context(tc.tile_pool(name="w", bufs=1))
    p["xc"] = ctx.enter_context(tc.tile_pool(name="xc", bufs=3))
    # PSUM: 8 banks. scores 2x[128,1024]=4, proj 2x[128,512]=2, av 1x[64,1024]=2
    p["sp"] = ctx.enter_context(tc.tile_pool(name="sp", bufs=2, space="PSUM"))
    p["pj"] = ctx.enter_context(tc.tile_pool(name="pj", bufs=2, space="PSUM"))
    p["av"] = ctx.enter_context(tc.tile_pool(name="av", bufs=1, space="PSUM"))
    p["qk"] = ctx.enter_context(tc.tile_pool(name="qk", bufs=12))
    p["v"] = ctx.enter_context(tc.tile_pool(name="v", bufs=2))
    p["probs"] = ctx.enter_context(tc.tile_pool(name="probs", bufs=18))
    p["o"] = ctx.enter_context(tc.tile_pool(name="o", bufs=5))
    p["y"] = ctx.enter_context(tc.tile_pool(name="y", bufs=2))
    p["sm"] = ctx.enter_context(tc.tile_pool(name="sm", bufs=8))
    p["vsc"] = ctx.enter_context(tc.tile_pool(name="vsc", bufs=3))
    return p


class Body:
    """One batch element's compute; emission interleave is driven outside."""

    def __init__(self, tc, io, p, w, bias, bodyi, variant):
        self.tc, self.io, self.p, self.bodyi = tc, io, p, bodyi
        self.variant = variant
        self.nc = tc.nc
        self.w = w          # WqT/WkT/WvT/WoT [128,4,512] fp16, bvr, ones
        self.bias = bias    # [128,12] f32: bq/8 | bk | bo columns
        self.ctxT = None
        self.xT = None
        self.K = [None] * 4
        self.Q = [None] * 4
        self.Vt = None
        self.O = [None] * 4
        self.Y = None
        self.probs = {}
        self.sums = {}
        self.vsc = {}

    def dma_in(self):
        p, nc, i = self.p, self.nc, self.bodyi
        xc = p["xc"].tile([128, XBLOB_F], F16, tag="xc", name=f"xc{i}")
        nc.sync.dma_start(out=xc[:], in_=self.io["xblob"])
        self.ctxT = xc[:, OFF_CTX:OFF_CTX + 4096]
        self.xT = xc[:, OFF_X:OFF_X + 4096]

    def _proj(self, dst_list, m, wT, src, bias_col, nm):
        """dst[m][p = inner chunk m, hw] = wT[:,kc,...]^T @ src + bias.
        Two hw-halves ping-pong through 1-bank PSUM tiles so the PE never
        waits on a full-tile eviction."""
        p, nc, i = self.p, self.nc, self.bodyi
        srcv = src.rearrange("p (kc f) -> p kc f", kc=4)
        dst = p["qk"].tile([128, 1024], F16, tag="qk", name=f"{nm}{m}_{i}")
        for n in range(2):
            ps = p["pj"].tile([128, 512], F32, tag="pj",
                              name=f"ps_{nm}{m}_{n}_{i}")
            for kc in range(4):
                nc.tensor.matmul(ps[:],
                                 wT[:, kc, m * 128:(m + 1) * 128],
                                 srcv[:, kc, n * 512:(n + 1) * 512],
                                 start=(kc == 0), stop=(kc == 3))
            nc.vector.tensor_scalar_add(dst[:, n * 512:(n + 1) * 512], ps[:],
                                        bias_col)
        dst_list[m] = dst

    def kq(self, m):
        self._proj(self.K, m, self.w["WkT"], self.ctxT,
                   self.bias[:, 4 + m:5 + m], "k")
        self._proj(self.Q, m, self.w["WqT"], self.xT,
                   self.bias[:, m:m + 1], "q")

    def vproj(self):
        """Vt[hw, jt, inner] in pairs: two hw-chunks per PSUM tile."""
        p, nc, i = self.p, self.nc, self.bodyi
        ctxv = self.ctxT.rearrange("p (kc f) -> p kc f", kc=4)
        self.Vt = p["v"].tile([128, 8, 512], ATT_DT, tag="v", name=f"vt_{i}")
        for jp in range(4):
            ps = p["sp"].tile([128, 1024], F32, tag="sp", name=f"ps_v{jp}_{i}")
            for half in range(2):
                jt = jp * 2 + half
                sl = ps[:, half * 512:(half + 1) * 512]
                for kc in range(4):
                    nc.tensor.matmul(sl, ctxv[:, kc, jt * 128:(jt + 1) * 128],
                                     self.w["WvT"][:, kc, :],
                                     start=(kc == 0), stop=False)
                nc.tensor.matmul(sl, self.w["ones"][:, 0:128],
                                 self.w["bvr"], start=False, stop=True)
            nc.vector.tensor_copy(
                out=self.Vt[:, jp * 2:jp * 2 + 2, :].rearrange("p a b -> p (a b)"),
                in_=ps[:])

    def scores_exp(self, h):
        p, nc, i = self.p, self.nc, self.bodyi
        m_h, p0 = h // 2, (h % 2) * 64
        Qh = self.Q[m_h][p0:p0 + 64, :]
        Kh = self.K[m_h][p0:p0 + 64, :]
        probs_l = []
        sums = p["sm"].tile([128, 8], F32, tag="sums", name=f"sums{h}_{i}")
        for qt in range(8):
            ps = p["sp"].tile([128, 1024], F32, tag="sp", name=f"ps_s{h}_{qt}_{i}")
            qslice = Qh[:, qt * 128:(qt + 1) * 128]
            nc.tensor.matmul(ps[:, 0:512], qslice, Kh[:, 0:512],
                             start=True, stop=True)
            nc.tensor.matmul(ps[:, 512:1024], qslice, Kh[:, 512:1024],
                             start=True, stop=True)
            probs = p["probs"].tile([128, 1024], ATT_DT, tag="probs",
                                    name=f"probs{h}_{qt}_{i}")
            if self.variant == "noexp":
                nc.scalar.copy(probs[:], ps[:])
            else:
                nc.scalar.activation(out=probs[:], in_=ps[:],
                                     func=mybir.ActivationFunctionType.Exp,
                                     accum_out=sums[:, qt:qt + 1])
            probs_l.append(probs)
        self.probs[h] = probs_l
        self.sums[h] = sums

    def recip_vsc(self, h):
        p, nc, i = self.p, self.nc, self.bodyi
        if self.variant == "noexp":
            return
        rec = p["sm"].tile([128, 8], F16, tag="rec", name=f"rec{h}_{i}")
        with nc.allow_low_precision(reason="softmax reciprocal"):
            nc.vector.reciprocal(out=rec[:], in_=self.sums[h][:])
        vsc_t = p["vsc"].tile([128, 8, 64], ATT_DT, tag="vsc", name=f"vsc{h}_{i}")
        rec_b = bass.AP(tensor=rec.tensor, offset=rec[:].offset,
                        ap=[rec[:].ap[0], rec[:].ap[1], [0, 64]])
        nc.vector.tensor_mul(vsc_t[:], self.Vt[:, :, h * 64:(h + 1) * 64],
                             rec_b)
        self.vsc[h] = vsc_t

    def av(self, h):
        p, nc, i = self.p, self.nc, self.bodyi
        m_h, p0 = h // 2, (h % 2) * 64
        po = p["av"].tile([64, 1024], F32, tag="av", name=f"po{h}_{i}")
        for qt in range(8):
            if self.variant == "noexp":
                vsc = self.Vt[:, qt, h * 64:(h + 1) * 64]
            else:
                vsc = self.vsc[h][:, qt, :]
            probs = self.probs[h][qt]
            nc.tensor.matmul(po[:, 0:512], vsc, probs[:, 0:512],
                             start=(qt == 0), stop=(qt == 7))
            nc.tensor.matmul(po[:, 512:1024], vsc, probs[:, 512:1024],
                             start=(qt == 0), stop=(qt == 7))
        if self.O[m_h] is None:
            self.O[m_h] = p["o"].tile([128, 1024], F16, tag="o",
                                      name=f"o{m_h}_{i}")
        nc.vector.tensor_copy(out=self.O[m_h][p0:p0 + 64, :], in_=po[:])
        del self.probs[h]
        self.vsc.pop(h, None)

    def yproj(self, m):
        p, nc, i = self.p, self.nc, self.bodyi
        if m == 0:
            self.Y = p["y"].tile([128, 4096], F16, tag="y", name=f"y_{i}")
        for n in range(2):
            ps = p["pj"].tile([128, 512], F32, tag="pj", name=f"ps_y{m}_{n}_{i}")
            for kc in range(4):
                nc.tensor.matmul(ps[:],
                                 self.w["WoT"][:, kc, m * 128:(m + 1) * 128],
                                 self.O[kc][:, n * 512:(n + 1) * 512],
                                 start=(kc == 0), stop=(kc == 3))
            nc.vector.tensor_scalar_add(
                self.Y[:, m * 1024 + n * 512:m * 1024 + (n + 1) * 512], ps[:],
                self.bias[:, 8 + m:9 + m])
        if m == 3:
            nc.scalar.dma_start(out=self.io["y"], in_=self.Y[:])


def _load_weights(tc, io, p):
    nc = tc.nc
    wt = p["w"].tile([128, WBLOB_F], F16, tag="wblob")
    nc.sync.dma_start(out=wt[:], in_=io["wblob"])
    bt = p["w"].tile([128, 12], F32, tag="biasf32")
    nc.vector.tensor_copy(out=bt[:], in_=wt[:, OFF_BIAS:OFF_BIAS + 12])

    def seg(off, ln):
        return wt[:, off:off + ln]

    w = {
        "WqT": seg(OFF_WQ, 2048).rearrange("p (kc f) -> p kc f", kc=4),
        "WkT": seg(OFF_WK, 2048).rearrange("p (kc f) -> p kc f", kc=4),
        "WvT": seg(OFF_WV, 2048).rearrange("p (kc f) -> p kc f", kc=4),
        "WoT": seg(OFF_WO, 2048).rearrange("p (kc f) -> p kc f", kc=4),
        "bvr": wt[0:1, OFF_BVR:OFF_BVR + 512],
        "ones": wt[0:1, OFF_ONES:OFF_ONES + 512],
    }
    return w, bt[:]


def _emit_mid(b: "Body", nxt: "Body | None"):
    """Everything after (kq0, s0) for body b, interleaved so ScalarE never
    drains; nxt's input DMA + kq0/s0 are pre-emitted before b's Y phase."""
    if nxt is not None:
        nxt.dma_in()
    b.kq(1)
    b.scores_exp(1)
    b.vproj()
    b.recip_vsc(0)
    b.av(0)
    b.kq(2)
    b.scores_exp(2)
    b.recip_vsc(1)
    b.av(1)
    b.kq(3)
    b.scores_exp(3)
    b.recip_vsc(2)
    b.av(2)
    b.scores_exp(4)
    b.recip_vsc(3)
    b.av(3)
    b.scores_exp(5)
    b.recip_vsc(4)
    b.av(4)
    b.scores_exp(6)
    b.recip_vsc(5)
    b.av(5)
    b.scores_exp(7)
    b.recip_vsc(6)
    b.av(6)
    if nxt is not None:
        nxt.kq(0)
        nxt.scores_exp(0)
    b.recip_vsc(7)
    b.av(7)
    for m in range(4):
        b.yproj(m)


def _emit_variant(b: "Body", nxt: "Body | None"):
    nc, p = b.nc, b.p
    if b.variant == "none":
        t = p["y"].tile([128, 16], F16, tag="noop", name=f"nop_{b.bodyi}")
        nc.vector.tensor_copy(out=t[:], in_=b.w["WqT"][:, 0, 0:16])
        return
    if b.variant == "dma":
        if nxt is not None:
            nxt.dma_in()
        y = p["y"].tile([128, 4096], F16, tag="y", name=f"yd_{b.bodyi}")
        nc.vector.tensor_copy(out=y[:], in_=b.ctxT[:])
        nc.scalar.dma_start(out=b.io["y"], in_=y[:])
        return
    if b.variant == "proj":
        if nxt is not None:
            nxt.dma_in()
        b.kq(0); b.kq(1); b.kq(2); b.kq(3)
        b.vproj()
        for m in range(4):
            b.O[m] = p["o"].tile([128, 1024], F16, tag="o",
                                 name=f"o{m}_{b.bodyi}")
            nc.vector.tensor_copy(out=b.O[m][:],
                                  in_=b.ctxT[:, m * 1024:(m + 1) * 1024])
        for m in range(4):
            b.yproj(m)
        return
    raise ValueError(b.variant)


def build_nc(repeat: int = 1, variant: str = "full"):
    nc = bacc.Bacc("TRN2", target_bir_lowering=False, debug=False)
    io = {
        "wblob": nc.dram_tensor("wblob", [128, WBLOB_F], F16,
                                kind="ExternalInput").ap(),
        "xblob": nc.dram_tensor("xblob", [128, XBLOB_F], F16,
                                kind="ExternalInput").ap(),
        "y": nc.dram_tensor("y", [128, 4096], F16,
                            kind="ExternalOutput").ap(),
    }
    with tile.TileContext(nc) as tc:
        with ExitStack() as ctx:
            p = make_pools(ctx, tc)
            w, bias = _load_weights(tc, io, p)
            bodies = [Body(tc, io, p, w, bias, i, variant)
                      for i in range(repeat)]
            bodies[0].dma_in()
            if variant in ("full", "noexp"):
                bodies[0].kq(0)
                bodies[0].scores_exp(0)
                for i in range(repeat):
                    _emit_mid(bodies[i],
                              bodies[i + 1] if i + 1 < repeat else None)
            else:
                for i in range(repeat):
                    _emit_variant(bodies[i],
                                  bodies[i + 1] if i + 1 < repeat else None)
    nc.compile()
    return nc


def _pack_cmajor(a: np.ndarray, nchunk: int) -> np.ndarray:
    """[nchunk*128, F] -> [128, nchunk*F] with row r = chunk*128 + p."""
    f = a.shape[1]
    return a.reshape(nchunk, 128, f).transpose(1, 0, 2).reshape(128, nchunk * f)


def make_in_maps(x, context, Wq, bq, Wk, bk, Wv, bv, Wo, bo):
    wblob = np.zeros((128, WBLOB_F), np.float16)
    wblob[:, OFF_WQ:OFF_WQ + 2048] = _pack_cmajor(
        np.ascontiguousarray(Wq.T) / 8.0, 4).astype(np.float16)
    wblob[:, OFF_WK:OFF_WK + 2048] = _pack_cmajor(
        np.ascontiguousarray(Wk.T), 4).astype(np.float16)
    wblob[:, OFF_WV:OFF_WV + 2048] = _pack_cmajor(
        np.ascontiguousarray(Wv.T), 4).astype(np.float16)
    wblob[:, OFF_WO:OFF_WO + 2048] = _pack_cmajor(
        np.ascontiguousarray(Wo.T), 4).astype(np.float16)
    wblob[0, OFF_BVR:OFF_BVR + 512] = bv.astype(np.float16)
    wblob[0, OFF_ONES:OFF_ONES + 512] = 1.0
    bias = np.empty((128, 12), np.float32)
    bias[:, 0:4] = (bq / 8.0).reshape(4, 128).T
    bias[:, 4:8] = bk.reshape(4, 128).T
    bias[:, 8:12] = bo.reshape(4, 128).T
    wblob[:, OFF_BIAS:OFF_BIAS + 12] = bias.astype(np.float16)

    in_maps = []
    for b in range(BS):
        xblob = np.empty((128, XBLOB_F), np.float16)
        xblob[:, OFF_CTX:OFF_CTX + 4096] = _pack_cmajor(
            context[b].reshape(C, HW), 4).astype(np.float16)
        xblob[:, OFF_X:OFF_X + 4096] = _pack_cmajor(
            x[b].reshape(C, HW), 4).astype(np.float16)
        in_maps.append({"wblob": wblob, "xblob": xblob})
    return in_maps


def unpack_y(y: np.ndarray) -> np.ndarray:
    # y[p, m*1024+hw] with C = m*128 + p
    return (y.astype(np.float32).reshape(128, 4, 1024)
            .transpose(1, 0, 2).reshape(C, H, W))


def kernel_with_results(inputs: dict, trace: bool = False, **run_kwargs):
    in_maps = make_in_maps(**{k: np.asarray(v, np.float32)
                              for k, v in inputs.items()})
    nc = build_nc()
    res = run_bass_kernel_spmd(nc, in_maps, core_ids=list(range(N_CORES)),
                               trace=trace, **run_kwargs)
    outs = [unpack_y(r["y"]) for r in res.results]
    return np.stack(outs), res


def kernel(**inputs) -> np.ndarray:
    out, _ = kernel_with_results(inputs)
    return out


# revision 3
# speedup vs baseline: 1.3540x; 1.3540x over previous
"""Trainium2 Bass kernel for nn_CrossAttention2 (8 cores, data-parallel over batch).

V4: fully fused software pipeline. ScalarE's 64-exp stream (~79us/body) is the
floor; the PE stream is emitted so that every exp's scores land just in time
and all other matmul work (AV of three heads back, Q/K/V projections in
half-tile slices, the previous body's output projection) fills the PE bubbles
between paced score matmuls. The PE executes in emission order, so the
schedule below IS the PE program.

Per body: 8 "phases" (one per head h). Phase h emits, per query-tile qt:
    scores mm (h, qt) x2 -> 1 of 2 ping-pong PSUM score tiles
    AV mm (head h-3, qt) x2 -> two 1-bank PSUM accumulator halves
    one background block at qt in {1,3,5,7}:
         kq half-projections / vproj halves / prev body's Y halves
    exp (h, qt) PSUM->SBUF fp16 + f32 row-sum accum [ScalarE]
AV lags 3 heads; heads 5..7's AV and the whole Y projection spill into the
NEXT body's phases 0..7, so the exp stream never drains at a body boundary.

IO: one 2MB fp16 input DMA per body (SP HWDGE queue, issued a body ahead),
one 1MB fp16 output DMA per body (Activation HWDGE queue), weights/biases
DMA'd once per NEFF. Softmax normalization is folded into V^T rows; Q/K/Y
biases ride the DVE PSUM->SBUF evictions (f32 per-partition columns); V's
bias is a rank-1 PE update.
"""

import os
import numpy as np
from contextlib import ExitStack

import concourse.bass as bass
from concourse import bacc
import concourse.tile as tile
from concourse import mybir
from concourse.bass_utils import run_bass_kernel_spmd

F32 = mybir.dt.float32
F16 = mybir.dt.float16
F32R = mybir.dt.float32r
ATT_DT = {"f16": F16, "f32r": F32R}[os.environ.get("ATT_DT", "f16")]

BS, C, H, W = 8, 512, 32, 32
HW = H * W
N_HEADS, DIM_HEAD = 8, 64
INNER = N_HEADS * DIM_HEAD
N_CORES = 8
AV_LAG = 3

OFF_WQ = 0
OFF_WK = 2048
OFF_WV = 4096
OFF_WO = 6144
OFF_BVR = 8192
OFF_ONES = 8704
OFF_BIAS = 9216     # fp16 [128,12] bq/8|bk|bo column-form biases
WBLOB_F = 9228
OFF_CTX = 0
OFF_X = 4096
XBLOB_F = 8192


def make_pools(ctx: ExitStack, tc: tile.TileContext):
    p = {}
    p["w"] = ctx.enter_context(tc.tile_pool(name="w", bufs=1))
    p["xc"] = ctx.enter_context(tc.tile_pool(name="xc", bufs=2))
    # PSUM: 8 banks = scores 2x[128,1024] + proj 2x[128,512] + av 2x[64,512]
    p["sp"] = ctx.enter_context(tc.tile_pool(name="sp", bufs=2, space="PSUM"))
    p["pj"] = ctx.enter_context(tc.tile_pool(name="pj", bufs=2, space="PSUM"))
    p["av"] = ctx.enter_context(tc.tile_pool(name="av", bufs=2, space="PSUM"))
    p["qk"] = ctx.enter_context(tc.tile_pool(name="qk", bufs=12))
    p["v"] = ctx.enter_context(tc.tile_pool(name="v", bufs=2))
    p["probs"] = ctx.enter_context(tc.tile_pool(name="probs", bufs=34))
    p["o"] = ctx.enter_context(tc.tile_pool(name="o", bufs=10))
    p["y"] = ctx.enter_context(tc.tile_pool(name="y", bufs=2))
    p["sm"] = ctx.enter_context(tc.tile_pool(name="sm", bufs=8))
    p["vsc"] = ctx.enter_context(tc.tile_pool(name="vsc", bufs=4))
    return p


class Body:
    def __init__(self, tc, io, p, w, bias, bodyi):
        self.tc, self.io, self.p, self.bodyi = tc, io, p, bodyi
        self.nc = tc.nc
        self.w = w
        self.bias = bias
        self.ctxT = None
        self.xT = None
        self.ctxv = None
        self.K = [None] * 4
        self.Q = [None] * 4
        self.Vt = None
        self.O = [None] * 4
        self.Y = None
        self._ps = {}
        self.probs = {}
        self.sums = {}
        self.vsc = {}
        self._avh = {}

    def dma_in(self, engine=None):
        p, nc, i = self.p, self.nc, self.bodyi
        xc = p["xc"].tile([128, XBLOB_F], F16, tag="xc", name=f"xc{i}")
        (engine or nc.sync).dma_start(out=xc[:], in_=self.io["xblob"])
        self.ctxT = xc[:, OFF_CTX:OFF_CTX + 4096]
        self.xT = xc[:, OFF_X:OFF_X + 4096]
        self.ctxv = self.ctxT.rearrange("p (kc f) -> p kc f", kc=4)

    # ---- background blocks (each ~4-5 matmuls + one DVE op) ----
    def kq_half(self, proj, m, n):
        """proj in ('k','q'); half n of dst[m] = W^T @ src + bias."""
        p, nc, i = self.p, self.nc, self.bodyi
        wT, src, bcol, lst = (
            (self.w["WkT"], self.ctxT, self.bias[:, 4 + m:5 + m], self.K)
            if proj == "k" else
            (self.w["WqT"], self.xT, self.bias[:, m:m + 1], self.Q))
        if n == 0 and lst[m] is None:
            lst[m] = p["qk"].tile([128, 1024], F16, tag="qk",
                                  name=f"{proj}{m}_{i}")
        srcv = src.rearrange("p (kc f) -> p kc f", kc=4)
        ps = p["pj"].tile([128, 512], F32, tag="pj",
                          name=f"ps_{proj}{m}_{n}_{i}")
        for kc in range(4):
            nc.tensor.matmul(ps[:], wT[:, kc, m * 128:(m + 1) * 128],
                             srcv[:, kc, n * 512:(n + 1) * 512],
                             start=(kc == 0), stop=(kc == 3))
        nc.vector.tensor_scalar_add(lst[m][:, n * 512:(n + 1) * 512], ps[:],
                                    bcol)

    def vproj_half(self, jt):
        p, nc, i = self.p, self.nc, self.bodyi
        if self.Vt is None:
            self.Vt = p["v"].tile([128, 8, 512], ATT_DT, tag="v",
                                  name=f"vt_{i}")
        ps = p["pj"].tile([128, 512], F32, tag="pj", name=f"ps_v{jt}_{i}")
        for kc in range(4):
            nc.tensor.matmul(ps[:], self.ctxv[:, kc, jt * 128:(jt + 1) * 128],
                             self.w["WvT"][:, kc, :],
                             start=(kc == 0), stop=False)
        nc.tensor.matmul(ps[:], self.w["ones"][:, 0:128], self.w["bvr"],
                         start=False, stop=True)
        nc.vector.tensor_copy(out=self.Vt[:, jt, :], in_=ps[:])

    def y_half(self, m, n):
        p, nc, i = self.p, self.nc, self.bodyi
        if self.Y is None:
            self.Y = p["y"].tile([128, 4096], F16, tag="y", name=f"y_{i}")
        ps = p["pj"].tile([128, 512], F32, tag="pj", name=f"ps_y{m}_{n}_{i}")
        for kc in range(4):
            nc.tensor.matmul(ps[:],
                             self.w["WoT"][:, kc, m * 128:(m + 1) * 128],
                             self.O[kc][:, n * 512:(n + 1) * 512],
                             start=(kc == 0), stop=(kc == 3))
        nc.vector.tensor_scalar_add(
            self.Y[:, m * 1024 + n * 512:m * 1024 + (n + 1) * 512], ps[:],
            self.bias[:, 8 + m:9 + m])

    def y_dma(self):
        self.nc.scalar.dma_start(out=self.io["y"], in_=self.Y[:])

    # ---- per-qt fused pieces ----
    def s_mm(self, h, qt):
        p, nc, i = self.p, self.nc, self.bodyi
        m_h, p0 = h // 2, (h % 2) * 64
        Qh = self.Q[m_h][p0:p0 + 64, :]
        Kh = self.K[m_h][p0:p0 + 64, :]
        ps = p["sp"].tile([128, 1024], F32, tag="sp", name=f"ps_s{h}_{qt}_{i}")
        qs = Qh[:, qt * 128:(qt + 1) * 128]
        nc.tensor.matmul(ps[:, 0:512], qs, Kh[:, 0:512], start=True, stop=True)
        nc.tensor.matmul(ps[:, 512:1024], qs, Kh[:, 512:1024],
                         start=True, stop=True)
        self._ps[(h, qt)] = ps

    def exp(self, h, qt):
        p, nc, i = self.p, self.nc, self.bodyi
        if qt == 0:
            self.sums[h] = p["sm"].tile([128, 8], F32, tag="sums",
                                        name=f"sums{h}_{i}")
            self.probs[h] = []
        probs = p["probs"].tile([128, 1024], ATT_DT, tag="probs",
                                name=f"probs{h}_{qt}_{i}")
        nc.scalar.activation(out=probs[:], in_=self._ps.pop((h, qt))[:],
                             func=mybir.ActivationFunctionType.Exp,
                             accum_out=self.sums[h][:, qt:qt + 1])
        self.probs[h].append(probs)

    def rv(self, h):
        p, nc, i = self.p, self.nc, self.bodyi
        rec = p["sm"].tile([128, 8], F16, tag="rec", name=f"rec{h}_{i}")
        with nc.allow_low_precision(reason="softmax reciprocal"):
            nc.vector.reciprocal(out=rec[:], in_=self.sums[h][:])
        vsc_t = p["vsc"].tile([128, 8, 64], ATT_DT, tag="vsc",
                              name=f"vsc{h}_{i}")
        rec_b = bass.AP(tensor=rec.tensor, offset=rec[:].offset,
                        ap=[rec[:].ap[0], rec[:].ap[1], [0, 64]])
        nc.vector.tensor_mul(vsc_t[:], self.Vt[:, :, h * 64:(h + 1) * 64],
                             rec_b)
        self.vsc[h] = vsc_t

    def av_mm(self, h, qt):
        p, nc, i = self.p, self.nc, self.bodyi
        if qt == 0:
            self._avh[h] = [
                p["av"].tile([64, 512], F32, tag="av", name=f"po{h}_{k}_{i}")
                for k in range(2)]
        po = self._avh[h]
        vsc = self.vsc[h][:, qt, :]
        probs = self.probs[h][qt]
        nc.tensor.matmul(po[0][:], vsc, probs[:, 0:512],
                         start=(qt == 0), stop=(qt == 7))
        nc.tensor.matmul(po[1][:], vsc, probs[:, 512:1024],
                         start=(qt == 0), stop=(qt == 7))

    def av_out(self, h):
        p, nc, i = self.p, self.nc, self.bodyi
        m_h, p0 = h // 2, (h % 2) * 64
        if self.O[m_h] is None:
            self.O[m_h] = p["o"].tile([128, 1024], F16, tag="o",
                                      name=f"o{m_h}_{i}")
        po = self._avh.pop(h)
        nc.vector.tensor_copy(out=self.O[m_h][p0:p0 + 64, 0:512], in_=po[0][:])
        nc.vector.tensor_copy(out=self.O[m_h][p0:p0 + 64, 512:1024],
                              in_=po[1][:])
        del self.probs[h]
        del self.vsc[h]


def _bg_schedule(b, prev, nxt):
    """bg[phase] = list of emit-closures; fixed deadline-safe assignment.

    deadlines: kq(m) before phase 2m; vproj before rv0 at end of phase 2;
    prev.Y after prev.av7 (our phase 2); nxt.kq0 before nxt's phase 0.
    """
    bg = [[] for _ in range(8)]
    bg[0] = [lambda m=m, n=n: b.kq_half(m and "q" or "k", 1, n)
             for m, n in ((0, 0), (0, 1), (1, 0), (1, 1))]
    bg[0] += [lambda jt=jt: b.vproj_half(jt) for jt in (0, 1)]
    bg[1] = [lambda jt=jt: b.vproj_half(jt) for jt in (2, 3, 4)]
    bg[2] = [lambda jt=jt: b.vproj_half(jt) for jt in (5, 6, 7)]
    bg[3] = [lambda m=m, n=n: b.kq_half(m and "q" or "k", 2, n)
             for m, n in ((0, 0), (0, 1), (1, 0), (1, 1))]
    if prev is not None:
        bg[3].append(lambda: prev.y_half(0, 0))
        bg[4] = [lambda: prev.y_half(0, 1), lambda: prev.y_half(1, 0)]
        bg[5] = [lambda: prev.y_half(1, 1), lambda: prev.y_half(2, 0)]
        bg[6] = [lambda: prev.y_half(2, 1), lambda: prev.y_half(3, 0)]
        bg[7] = [lambda: prev.y_half(3, 1), lambda: prev.y_dma()]
    bg[4] += [lambda m=m, n=n: b.kq_half(m and "q" or "k", 3, n)
              for m, n in ((0, 0), (0, 1))]
    bg[5] += [lambda m=m, n=n: b.kq_half(m and "q" or "k", 3, n)
              for m, n in ((1, 0), (1, 1))]
    if nxt is not None:
        bg[6] += [lambda m=m, n=n: nxt.kq_half(m and "q" or "k", 0, n)
                  for m, n in ((0, 0), (0, 1))]
        bg[7] += [lambda m=m, n=n: nxt.kq_half(m and "q" or "k", 0, n)
                  for m, n in ((1, 0), (1, 1))]
    return bg


def _emit_body(b, prev, nxt):
    """8 fused phases. Phase h: scores/exp of head h, AV of head h-AV_LAG
    (negative -> prev body's head h-AV_LAG+8), background blocks between."""
    if nxt is not None:
        nxt.dma_in()
    bg = _bg_schedule(b, prev, nxt)
    for h in range(8):
        ah = h - AV_LAG            # AV task for this phase
        avb = b if ah >= 0 else prev
        if avb is not None and ah < 0:
            ah += 8
        blocks = list(bg[h])
        bi = 0
        for qt in range(8):
            b.s_mm(h, qt)
            if avb is not None:
                avb.av_mm(ah, qt)
            if qt % 2 == 1 and bi < len(blocks):
                blocks[bi]()
                bi += 1
            b.exp(h, qt)
        while bi < len(blocks):
            blocks[bi]()
            bi += 1
        if avb is not None:
            avb.av_out(ah)
        # reciprocal+vsc for head h-(AV_LAG-1) so its AV can start next phase
        rh = h - (AV_LAG - 1)
        rvb = b if rh >= 0 else prev
        if rvb is not None and rh < 0:
            rh += 8
        if rvb is not None:
            rvb.rv(rh)


def _emit_tail(b):
    """Drain body b (the last one): AV heads 5..7, Y projection, out-DMA."""
    for ah in (5, 6, 7):
        for qt in range(8):
            b.av_mm(ah, qt)
        b.av_out(ah)
        if ah < 7:
            b.rv(ah + 1)
    for m in range(4):
        b.y_half(m, 0)
        b.y_half(m, 1)
    b.y_dma()


def _load_weights(tc, io, p):
    # Wq|Wk + biases first so body 0's kq0 can start before Wv/Wo land.
    nc = tc.nc
    wt = p["w"].tile([128, WBLOB_F], F16, tag="wblob")
    nc.sync.dma_start(out=wt[:, 0:4096], in_=io["wblob"][:, 0:4096])
    nc.sync.dma_start(out=wt[:, OFF_BIAS:OFF_BIAS + 12],
                      in_=io["wblob"][:, OFF_BIAS:OFF_BIAS + 12])
    nc.sync.dma_start(out=wt[:, 4096:OFF_BIAS], in_=io["wblob"][:, 4096:OFF_BIAS])
    bt = p["w"].tile([128, 12], F32, tag="biasf32")
    nc.vector.tensor_copy(out=bt[:], in_=wt[:, OFF_BIAS:OFF_BIAS + 12])

    def seg(off, ln):
        return wt[:, off:off + ln]

    w = {
        "WqT": seg(OFF_WQ, 2048).rearrange("p (kc f) -> p kc f", kc=4),
        "WkT": seg(OFF_WK, 2048).rearrange("p (kc f) -> p kc f", kc=4),
        "WvT": seg(OFF_WV, 2048).rearrange("p (kc f) -> p kc f", kc=4),
        "WoT": seg(OFF_WO, 2048).rearrange("p (kc f) -> p kc f", kc=4),
        "bvr": wt[0:1, OFF_BVR:OFF_BVR + 512],
        "ones": wt[0:1, OFF_ONES:OFF_ONES + 512],
    }
    return w, bt[:]


def build_nc(repeat: int = 1, variant: str = "full"):
    assert variant == "full"
    nc = bacc.Bacc("TRN2", target_bir_lowering=False, debug=False)
    io = {
        "wblob": nc.dram_tensor("wblob", [128, WBLOB_F], F16,
                                kind="ExternalInput").ap(),
        "xblob": nc.dram_tensor("xblob", [128, XBLOB_F], F16,
                                kind="ExternalInput").ap(),
        "y": nc.dram_tensor("y", [128, 4096], F16,
                            kind="ExternalOutput").ap(),
    }
    with tile.TileContext(nc) as tc:
        with ExitStack() as ctx:
            p = make_pools(ctx, tc)
            w, bias = _load_weights(tc, io, p)
            bodies = [Body(tc, io, p, w, bias, i) for i in range(repeat)]
            b0 = bodies[0]
            b0.dma_in(engine=nc.scalar)  # ACT HWDGE queue, parallel with wblob
            for m, n in ((0, 0), (0, 1), (1, 0), (1, 1)):
                b0.kq_half("k" if m == 0 else "q", 0, n)
            for i in range(repeat):
                _emit_body(bodies[i],
                           bodies[i - 1] if i > 0 else None,
                           bodies[i + 1] if i + 1 < repeat else None)
            _emit_tail(bodies[-1])
    nc.compile()
    return nc


def _pack_cmajor(a: np.ndarray, nchunk: int) -> np.ndarray:
    f = a.shape[1]
    return a.reshape(nchunk, 128, f).transpose(1, 0, 2).reshape(128, nchunk * f)


def make_in_maps(x, context, Wq, bq, Wk, bk, Wv, bv, Wo, bo):
    wblob = np.zeros((128, WBLOB_F), np.float16)
    wblob[:, OFF_WQ:OFF_WQ + 2048] = _pack_cmajor(
        np.ascontiguousarray(Wq.T) / 8.0, 4).astype(np.float16)
    wblob[:, OFF_WK:OFF_WK + 2048] = _pack_cmajor(
        np.ascontiguousarray(Wk.T), 4).astype(np.float16)
    wblob[:, OFF_WV:OFF_WV + 2048] = _pack_cmajor(
        np.ascontiguousarray(Wv.T), 4).astype(np.float16)
    wblob[:, OFF_WO:OFF_WO + 2048] = _pack_cmajor(
        np.ascontiguousarray(Wo.T), 4).astype(np.float16)
    wblob[0, OFF_BVR:OFF_BVR + 512] = bv.astype(np.float16)
    wblob[0, OFF_ONES:OFF_ONES + 512] = 1.0
    bias = np.empty((128, 12), np.float32)
    bias[:, 0:4] = (bq / 8.0).reshape(4, 128).T
    bias[:, 4:8] = bk.reshape(4, 128).T
    bias[:, 8:12] = bo.reshape(4, 128).T
    wblob[:, OFF_BIAS:OFF_BIAS + 12] = bias.astype(np.float16)

    in_maps = []
    for b in range(BS):
        xblob = np.empty((128, XBLOB_F), np.float16)
        xblob[:, OFF_CTX:OFF_CTX + 4096] = _pack_cmajor(
            context[b].reshape(C, HW), 4).astype(np.float16)
        xblob[:, OFF_X:OFF_X + 4096] = _pack_cmajor(
            x[b].reshape(C, HW), 4).astype(np.float16)
        in_maps.append({"wblob": wblob, "xblob": xblob})
    return in_maps


def unpack_y(y: np.ndarray) -> np.ndarray:
    return (y.astype(np.float32).reshape(128, 4, 1024)
            .transpose(1, 0, 2).reshape(C, H, W))


def kernel_with_results(inputs: dict, trace: bool = False, **run_kwargs):
    in_maps = make_in_maps(**{k: np.asarray(v, np.float32)
                              for k, v in inputs.items()})
    nc = build_nc()
    res = run_bass_kernel_spmd(nc, in_maps, core_ids=list(range(N_CORES)),
                               trace=trace, **run_kwargs)
    outs = [unpack_y(r["y"]) for r in res.results]
    return np.stack(outs), res


def kernel(**inputs) -> np.ndarray:
    out, _ = kernel_with_results(inputs)
    return out


# revision 4
# speedup vs baseline: 1.4998x; 1.1077x over previous
"""Trainium2 Bass kernel for nn_CrossAttention2 (8 cores, data-parallel over batch).

V4: fully fused software pipeline. ScalarE's 64-exp stream (~79us/body) is the
floor; the PE stream is emitted so that every exp's scores land just in time
and all other matmul work (AV of three heads back, Q/K/V projections in
half-tile slices, the previous body's output projection) fills the PE bubbles
between paced score matmuls. The PE executes in emission order, so the
schedule below IS the PE program.

Per body: 8 "phases" (one per head h). Phase h emits, per query-tile qt:
    scores mm (h, qt) x2 -> 1 of 2 ping-pong PSUM score tiles
    AV mm (head h-3, qt) x2 -> two 1-bank PSUM accumulator halves
    one background block at qt in {1,3,5,7}:
         kq half-projections / vproj halves / prev body's Y halves
    exp (h, qt) PSUM->SBUF fp16 + f32 row-sum accum [ScalarE]
AV lags 3 heads; heads 5..7's AV and the whole Y projection spill into the
NEXT body's phases 0..7, so the exp stream never drains at a body boundary.

IO: one 2MB fp16 input DMA per body (SP HWDGE queue, issued a body ahead),
one 1MB fp16 output DMA per body (Activation HWDGE queue), weights/biases
DMA'd once per NEFF. Softmax normalization is folded into V^T rows; Q/K/Y
biases ride the DVE PSUM->SBUF evictions (f32 per-partition columns); V's
bias is a rank-1 PE update.
"""

import os
import numpy as np
from contextlib import ExitStack

import concourse.bass as bass
from concourse import bacc
import concourse.tile as tile
from concourse import mybir
from concourse.bass_utils import run_bass_kernel_spmd

F32 = mybir.dt.float32
F16 = mybir.dt.float16
F32R = mybir.dt.float32r
ATT_DT = {"f16": F16, "f32r": F32R}[os.environ.get("ATT_DT", "f16")]

BS, C, H, W = 8, 512, 32, 32
HW = H * W
N_HEADS, DIM_HEAD = 8, 64
INNER = N_HEADS * DIM_HEAD
N_CORES = 8
AV_LAG = 3

OFF_WQ = 0
OFF_WK = 2048
OFF_WV = 4096
OFF_WO = 6144
OFF_BVR = 8192
OFF_ONES = 8704
OFF_BIAS = 9216     # fp16 [128,12] bq/8|bk|bo column-form biases
WBLOB_F = 9228
OFF_CTX = 0
OFF_X = 4096
XBLOB_F = 8192


def make_pools(ctx: ExitStack, tc: tile.TileContext):
    p = {}
    p["w"] = ctx.enter_context(tc.tile_pool(name="w", bufs=1))
    p["xc"] = ctx.enter_context(tc.tile_pool(name="xc", bufs=2))
    # PSUM: 8 banks = scores 2x[128,1024] + proj 2x[128,512] + av 2x[64,512]
    p["sp"] = ctx.enter_context(tc.tile_pool(name="sp", bufs=2, space="PSUM"))
    p["pj"] = ctx.enter_context(tc.tile_pool(name="pj", bufs=2, space="PSUM"))
    p["av"] = ctx.enter_context(tc.tile_pool(name="av", bufs=2, space="PSUM"))
    p["qk"] = ctx.enter_context(tc.tile_pool(name="qk", bufs=12))
    p["v"] = ctx.enter_context(tc.tile_pool(name="v", bufs=2))
    p["probs"] = ctx.enter_context(tc.tile_pool(name="probs", bufs=34))
    p["o"] = ctx.enter_context(tc.tile_pool(name="o", bufs=10))
    p["y"] = ctx.enter_context(tc.tile_pool(name="y", bufs=2))
    p["sm"] = ctx.enter_context(tc.tile_pool(name="sm", bufs=8))
    p["vsc"] = ctx.enter_context(tc.tile_pool(name="vsc", bufs=4))
    return p


class Body:
    def __init__(self, tc, io, p, w, bias, bodyi):
        self.tc, self.io, self.p, self.bodyi = tc, io, p, bodyi
        self.nc = tc.nc
        self.w = w
        self.bias = bias
        self.ctxT = None
        self.xT = None
        self.ctxv = None
        self.K = [None] * 4
        self.Q = [None] * 4
        self.Vt = None
        self.O = [None] * 4
        self.Y = None
        self._ps = {}
        self.probs = {}
        self.sums = {}
        self.vsc = {}
        self._avh = {}

    def dma_in(self, engine=None):
        p, nc, i = self.p, self.nc, self.bodyi
        xc = p["xc"].tile([128, XBLOB_F], F16, tag="xc", name=f"xc{i}")
        (engine or nc.sync).dma_start(out=xc[:], in_=self.io["xblob"])
        self.ctxT = xc[:, OFF_CTX:OFF_CTX + 4096]
        self.xT = xc[:, OFF_X:OFF_X + 4096]
        self.ctxv = self.ctxT.rearrange("p (kc f) -> p kc f", kc=4)

    # ---- background blocks (each ~4-5 matmuls + one DVE op) ----
    def kq_half(self, proj, m, n):
        """proj in ('k','q'); half n of dst[m] = W^T @ src + bias."""
        p, nc, i = self.p, self.nc, self.bodyi
        wT, src, bcol, lst = (
            (self.w["WkT"], self.ctxT, self.bias[:, 4 + m:5 + m], self.K)
            if proj == "k" else
            (self.w["WqT"], self.xT, self.bias[:, m:m + 1], self.Q))
        if n == 0 and lst[m] is None:
            lst[m] = p["qk"].tile([128, 1024], F16, tag="qk",
                                  name=f"{proj}{m}_{i}")
        srcv = src.rearrange("p (kc f) -> p kc f", kc=4)
        ps = p["pj"].tile([128, 512], F32, tag="pj",
                          name=f"ps_{proj}{m}_{n}_{i}")
        for kc in range(4):
            nc.tensor.matmul(ps[:], wT[:, kc, m * 128:(m + 1) * 128],
                             srcv[:, kc, n * 512:(n + 1) * 512],
                             start=(kc == 0), stop=(kc == 3))
        nc.vector.tensor_scalar_add(lst[m][:, n * 512:(n + 1) * 512], ps[:],
                                    bcol)

    def vproj_half(self, jt):
        p, nc, i = self.p, self.nc, self.bodyi
        if self.Vt is None:
            self.Vt = p["v"].tile([128, 8, 512], ATT_DT, tag="v",
                                  name=f"vt_{i}")
        ps = p["pj"].tile([128, 512], F32, tag="pj", name=f"ps_v{jt}_{i}")
        for kc in range(4):
            nc.tensor.matmul(ps[:], self.ctxv[:, kc, jt * 128:(jt + 1) * 128],
                             self.w["WvT"][:, kc, :],
                             start=(kc == 0), stop=False)
        nc.tensor.matmul(ps[:], self.w["ones"][:, 0:128], self.w["bvr"],
                         start=False, stop=True)
        nc.vector.tensor_copy(out=self.Vt[:, jt, :], in_=ps[:])

    def y_half(self, m, n):
        p, nc, i = self.p, self.nc, self.bodyi
        if self.Y is None:
            self.Y = p["y"].tile([128, 4096], F16, tag="y", name=f"y_{i}")
        ps = p["pj"].tile([128, 512], F32, tag="pj", name=f"ps_y{m}_{n}_{i}")
        for kc in range(4):
            nc.tensor.matmul(ps[:],
                             self.w["WoT"][:, kc, m * 128:(m + 1) * 128],
                             self.O[kc][:, n * 512:(n + 1) * 512],
                             start=(kc == 0), stop=(kc == 3))
        nc.vector.tensor_scalar_add(
            self.Y[:, m * 1024 + n * 512:m * 1024 + (n + 1) * 512], ps[:],
            self.bias[:, 8 + m:9 + m])

    def y_dma(self):
        self.nc.sync.dma_start(out=self.io["y"], in_=self.Y[:])

    # ---- per-qt fused pieces ----
    def s_mm(self, h, qt):
        p, nc, i = self.p, self.nc, self.bodyi
        m_h, p0 = h // 2, (h % 2) * 64
        Qh = self.Q[m_h][p0:p0 + 64, :]
        Kh = self.K[m_h][p0:p0 + 64, :]
        ps = p["sp"].tile([128, 1024], F32, tag="sp", name=f"ps_s{h}_{qt}_{i}")
        qs = Qh[:, qt * 128:(qt + 1) * 128]
        nc.tensor.matmul(ps[:, 0:512], qs, Kh[:, 0:512], start=True, stop=True)
        nc.tensor.matmul(ps[:, 512:1024], qs, Kh[:, 512:1024],
                         start=True, stop=True)
        self._ps[(h, qt)] = ps

    def exp(self, h, qt):
        p, nc, i = self.p, self.nc, self.bodyi
        if qt == 0:
            self.sums[h] = p["sm"].tile([128, 8], F32, tag="sums",
                                        name=f"sums{h}_{i}")
            self.probs[h] = []
        probs = p["probs"].tile([128, 1024], ATT_DT, tag="probs",
                                name=f"probs{h}_{qt}_{i}")
        nc.scalar.activation(out=probs[:], in_=self._ps.pop((h, qt))[:],
                             func=mybir.ActivationFunctionType.Exp,
                             accum_out=self.sums[h][:, qt:qt + 1])
        self.probs[h].append(probs)

    def rv(self, h):
        p, nc, i = self.p, self.nc, self.bodyi
        rec = p["sm"].tile([128, 8], F16, tag="rec", name=f"rec{h}_{i}")
        with nc.allow_low_precision(reason="softmax reciprocal"):
            nc.vector.reciprocal(out=rec[:], in_=self.sums[h][:])
        vsc_t = p["vsc"].tile([128, 8, 64], ATT_DT, tag="vsc",
                              name=f"vsc{h}_{i}")
        rec_b = bass.AP(tensor=rec.tensor, offset=rec[:].offset,
                        ap=[rec[:].ap[0], rec[:].ap[1], [0, 64]])
        nc.vector.tensor_mul(vsc_t[:], self.Vt[:, :, h * 64:(h + 1) * 64],
                             rec_b)
        self.vsc[h] = vsc_t

    def av_mm(self, h, qt):
        p, nc, i = self.p, self.nc, self.bodyi
        if qt == 0:
            self._avh[h] = [
                p["av"].tile([64, 512], F32, tag="av", name=f"po{h}_{k}_{i}")
                for k in range(2)]
        po = self._avh[h]
        vsc = self.vsc[h][:, qt, :]
        probs = self.probs[h][qt]
        nc.tensor.matmul(po[0][:], vsc, probs[:, 0:512],
                         start=(qt == 0), stop=(qt == 7))
        nc.tensor.matmul(po[1][:], vsc, probs[:, 512:1024],
                         start=(qt == 0), stop=(qt == 7))

    def av_out(self, h):
        p, nc, i = self.p, self.nc, self.bodyi
        m_h, p0 = h // 2, (h % 2) * 64
        if self.O[m_h] is None:
            self.O[m_h] = p["o"].tile([128, 1024], F16, tag="o",
                                      name=f"o{m_h}_{i}")
        po = self._avh.pop(h)
        nc.vector.tensor_copy(out=self.O[m_h][p0:p0 + 64, 0:512], in_=po[0][:])
        nc.vector.tensor_copy(out=self.O[m_h][p0:p0 + 64, 512:1024],
                              in_=po[1][:])
        del self.probs[h]
        del self.vsc[h]


def _bg_schedule(b, prev, nxt):
    """bg[phase] = list of emit-closures; fixed deadline-safe assignment.

    deadlines: kq(m) before phase 2m; vproj before rv0 at end of phase 2;
    prev.Y after prev.av7 (our phase 2); nxt.kq0 before nxt's phase 0.
    """
    bg = [[] for _ in range(8)]
    bg[0] = [lambda m=m, n=n: b.kq_half(m and "q" or "k", 1, n)
             for m, n in ((0, 0), (0, 1), (1, 0), (1, 1))]
    bg[0] += [lambda jt=jt: b.vproj_half(jt) for jt in (0, 1)]
    bg[1] = [lambda jt=jt: b.vproj_half(jt) for jt in (2, 3, 4)]
    bg[2] = [lambda jt=jt: b.vproj_half(jt) for jt in (5, 6, 7)]
    bg[3] = [lambda m=m, n=n: b.kq_half(m and "q" or "k", 2, n)
             for m, n in ((0, 0), (0, 1), (1, 0), (1, 1))]
    if prev is not None:
        bg[3].append(lambda: prev.y_half(0, 0))
        bg[4] = [lambda: prev.y_half(0, 1), lambda: prev.y_half(1, 0)]
        bg[5] = [lambda: prev.y_half(1, 1), lambda: prev.y_half(2, 0)]
        bg[6] = [lambda: prev.y_half(2, 1), lambda: prev.y_half(3, 0)]
        bg[7] = [lambda: prev.y_half(3, 1), lambda: prev.y_dma()]
    bg[4] += [lambda m=m, n=n: b.kq_half(m and "q" or "k", 3, n)
              for m, n in ((0, 0), (0, 1))]
    bg[5] += [lambda m=m, n=n: b.kq_half(m and "q" or "k", 3, n)
              for m, n in ((1, 0), (1, 1))]
    if nxt is not None:
        bg[6] += [lambda m=m, n=n: nxt.kq_half(m and "q" or "k", 0, n)
                  for m, n in ((0, 0), (0, 1))]
        bg[7] += [lambda m=m, n=n: nxt.kq_half(m and "q" or "k", 0, n)
                  for m, n in ((1, 0), (1, 1))]
    return bg


def _emit_body(b, prev, nxt):
    """8 fused phases. Phase h: scores/exp of head h, AV of head h-AV_LAG
    (negative -> prev body's head h-AV_LAG+8), background blocks between."""
    if nxt is not None:
        nxt.dma_in()
    bg = _bg_schedule(b, prev, nxt)
    for h in range(8):
        ah = h - AV_LAG            # AV task for this phase
        avb = b if ah >= 0 else prev
        if avb is not None and ah < 0:
            ah += 8
        blocks = list(bg[h])
        bi = 0
        for qt in range(8):
            b.s_mm(h, qt)
            if avb is not None:
                avb.av_mm(ah, qt)
            if qt % 2 == 1 and bi < len(blocks):
                blocks[bi]()
                bi += 1
            b.exp(h, qt)
        while bi < len(blocks):
            blocks[bi]()
            bi += 1
        if avb is not None:
            avb.av_out(ah)
        # reciprocal+vsc for head h-(AV_LAG-1) so its AV can start next phase
        rh = h - (AV_LAG - 1)
        rvb = b if rh >= 0 else prev
        if rvb is not None and rh < 0:
            rh += 8
        if rvb is not None:
            rvb.rv(rh)


def _emit_tail(b):
    """Drain body b (the last one): AV heads 5..7, Y projection, out-DMA."""
    for ah in (5, 6, 7):
        for qt in range(8):
            b.av_mm(ah, qt)
        b.av_out(ah)
        if ah < 7:
            b.rv(ah + 1)
    for m in range(4):
        b.y_half(m, 0)
        b.y_half(m, 1)
    b.y_dma()


def _load_weights(tc, io, p):
    # Wq|Wk + biases first so body 0's kq0 can start before Wv/Wo land.
    nc = tc.nc
    wt = p["w"].tile([128, WBLOB_F], F16, tag="wblob")
    nc.sync.dma_start(out=wt[:, 0:4096], in_=io["wblob"][:, 0:4096])
    nc.sync.dma_start(out=wt[:, OFF_BIAS:OFF_BIAS + 12],
                      in_=io["wblob"][:, OFF_BIAS:OFF_BIAS + 12])
    nc.sync.dma_start(out=wt[:, 4096:OFF_BIAS], in_=io["wblob"][:, 4096:OFF_BIAS])
    bt = p["w"].tile([128, 12], F32, tag="biasf32")
    nc.vector.tensor_copy(out=bt[:], in_=wt[:, OFF_BIAS:OFF_BIAS + 12])

    def seg(off, ln):
        return wt[:, off:off + ln]

    w = {
        "WqT": seg(OFF_WQ, 2048).rearrange("p (kc f) -> p kc f", kc=4),
        "WkT": seg(OFF_WK, 2048).rearrange("p (kc f) -> p kc f", kc=4),
        "WvT": seg(OFF_WV, 2048).rearrange("p (kc f) -> p kc f", kc=4),
        "WoT": seg(OFF_WO, 2048).rearrange("p (kc f) -> p kc f", kc=4),
        "bvr": wt[0:1, OFF_BVR:OFF_BVR + 512],
        "ones": wt[0:1, OFF_ONES:OFF_ONES + 512],
    }
    return w, bt[:]


def build_nc(repeat: int = 1, variant: str = "full"):
    assert variant == "full"
    nc = bacc.Bacc("TRN2", target_bir_lowering=False, debug=False)
    io = {
        "wblob": nc.dram_tensor("wblob", [128, WBLOB_F], F16,
                                kind="ExternalInput").ap(),
        "xblob": nc.dram_tensor("xblob", [128, XBLOB_F], F16,
                                kind="ExternalInput").ap(),
        "y": nc.dram_tensor("y", [128, 4096], F16,
                            kind="ExternalOutput").ap(),
    }
    with tile.TileContext(nc) as tc:
        with ExitStack() as ctx:
            p = make_pools(ctx, tc)
            w, bias = _load_weights(tc, io, p)
            bodies = [Body(tc, io, p, w, bias, i) for i in range(repeat)]
            b0 = bodies[0]
            b0.dma_in(engine=nc.scalar)  # ACT HWDGE queue, parallel with wblob
            for m, n in ((0, 0), (0, 1), (1, 0), (1, 1)):
                b0.kq_half("k" if m == 0 else "q", 0, n)
            for i in range(repeat):
                _emit_body(bodies[i],
                           bodies[i - 1] if i > 0 else None,
                           bodies[i + 1] if i + 1 < repeat else None)
            _emit_tail(bodies[-1])
    nc.compile()
    return nc


def _pack_cmajor(a: np.ndarray, nchunk: int) -> np.ndarray:
    f = a.shape[1]
    return a.reshape(nchunk, 128, f).transpose(1, 0, 2).reshape(128, nchunk * f)


def make_in_maps(x, context, Wq, bq, Wk, bk, Wv, bv, Wo, bo):
    wblob = np.zeros((128, WBLOB_F), np.float16)
    wblob[:, OFF_WQ:OFF_WQ + 2048] = _pack_cmajor(
        np.ascontiguousarray(Wq.T) / 8.0, 4).astype(np.float16)
    wblob[:, OFF_WK:OFF_WK + 2048] = _pack_cmajor(
        np.ascontiguousarray(Wk.T), 4).astype(np.float16)
    wblob[:, OFF_WV:OFF_WV + 2048] = _pack_cmajor(
        np.ascontiguousarray(Wv.T), 4).astype(np.float16)
    wblob[:, OFF_WO:OFF_WO + 2048] = _pack_cmajor(
        np.ascontiguousarray(Wo.T), 4).astype(np.float16)
    wblob[0, OFF_BVR:OFF_BVR + 512] = bv.astype(np.float16)
    wblob[0, OFF_ONES:OFF_ONES + 512] = 1.0
    bias = np.empty((128, 12), np.float32)
    bias[:, 0:4] = (bq / 8.0).reshape(4, 128).T
    bias[:, 4:8] = bk.reshape(4, 128).T
    bias[:, 8:12] = bo.reshape(4, 128).T
    wblob[:, OFF_BIAS:OFF_BIAS + 12] = bias.astype(np.float16)

    in_maps = []
    for b in range(BS):
        xblob = np.empty((128, XBLOB_F), np.float16)
        xblob[:, OFF_CTX:OFF_CTX + 4096] = _pack_cmajor(
            context[b].reshape(C, HW), 4).astype(np.float16)
        xblob[:, OFF_X:OFF_X + 4096] = _pack_cmajor(
            x[b].reshape(C, HW), 4).astype(np.float16)
        in_maps.append({"wblob": wblob, "xblob": xblob})
    return in_maps


def unpack_y(y: np.ndarray) -> np.ndarray:
    return (y.astype(np.float32).reshape(128, 4, 1024)
            .transpose(1, 0, 2).reshape(C, H, W))


def kernel_with_results(inputs: dict, trace: bool = False, **run_kwargs):
    in_maps = make_in_maps(**{k: np.asarray(v, np.float32)
                              for k, v in inputs.items()})
    nc = build_nc()
    res = run_bass_kernel_spmd(nc, in_maps, core_ids=list(range(N_CORES)),
                               trace=trace, **run_kwargs)
    outs = [unpack_y(r["y"]) for r in res.results]
    return np.stack(outs), res


def kernel(**inputs) -> np.ndarray:
    out, _ = kernel_with_results(inputs)
    return out
